# revision 5
# baseline (speedup 1.0000x reference)
"""LoRA cross-attention (self-attn) processor on 8 TRN2 NeuronCores.

Problem: B=4, S=2048, D=640, H=8 heads (hd=80), LoRA rank 4.
  q/k/v = x @ (W + up@down).T ; per-head attention; out = attn @ (Wo + o_up@o_down).T + bo

Sharding: batch*head parallel. Core c -> batch b=c//2, head-group g=c%2
(4 heads). Each core computes q/k/v projections for its 4 heads, the
attention for those heads, and a partial output projection (contraction
over its 320 head-dims). Host folds the rank-4 LoRA updates into the
weights (exact algebra), pre-transposes/casts operands, and sums the two
partial outputs per batch + bias at the end.

On-core layout (all matmul operands bf16, PSUM accumulation fp32):
  xT   [640, 2048]  x transposed (contraction dim on partitions)
  w_qk [640, 640]   cols = 4 q-heads * 80 dims then 4 k-heads * 80
  w_v  [640, 320]
  w_o  [320, 640]   rows = 4 heads * 80 dims
  outT [640, 2048]  fp32 partial output (pre-bias)

Attention per (head, q-chunk of 512): scoresT[k,q] tiles via PE,
exp on ACT (scale folded in, no max subtraction — scores are O(1) by
construction so fp32 exp is safe), PV with v as stationary operand and
an appended ones column producing the softmax denominators, then a
reciprocal + PE-broadcast + DVE multiply to normalize.
"""
import numpy as np
import ml_dtypes

B, S, D, H, HD, R = 4, 2048, 640, 8, 80, 4
HPC = H // 2          # heads per core
GDIM = HPC * HD       # 320 head-dims per core
NCORES = 8
NKT = S // 128        # 16 key tiles
NQC = S // 512        # 4 query chunks
NCT = D // 128        # 5 contraction tiles
SM_SCALE = 1.0 / float(np.sqrt(HD))

_cache = {}


def _body(tc, xT, w_qk, w_v, w_o, outT):
    import concourse.mybir as mybir

    nc = tc.nc
    bf = mybir.dt.bfloat16
    f32 = mybir.dt.float32
    Exp = mybir.ActivationFunctionType.Exp

    with tc.tile_pool(name="weights", bufs=1) as wpool, \
         tc.tile_pool(name="persist", bufs=1) as pers:
        wqk_t = []
        for i in range(NCT):
            t = wpool.tile([128, 2 * GDIM], bf, name=f"wqk{i}", tag=f"wqk{i}")
            nc.sync.dma_start(out=t, in_=w_qk[128 * i:128 * (i + 1), :])
            wqk_t.append(t)
        wv_t = []
        for i in range(NCT):
            t = wpool.tile([128, GDIM], bf, name=f"wv{i}", tag=f"wv{i}")
            nc.sync.dma_start(out=t, in_=w_v[128 * i:128 * (i + 1), :])
            wv_t.append(t)
        wo_t = []
        for h in range(HPC):
            t = wpool.tile([HD, D], bf, name=f"wo{h}", tag=f"wo{h}")
            nc.sync.dma_start(out=t, in_=w_o[HD * h:HD * (h + 1), :])
            wo_t.append(t)
        ones_bf = pers.tile([1, HD], bf, name="ones", tag="ones")
        nc.vector.memset(ones_bf, 1.0)

        qkT = [pers.tile([HD, S], bf, name=f"qkT{i}", tag=f"qkT{i}") for i in range(2 * HPC)]
        vaug = [[pers.tile([128, HD + 1], bf, name=f"va{h}_{k}", tag=f"va{h}_{k}") for k in range(NKT)]
                for h in range(HPC)]
        for h in range(HPC):
            for k in range(NKT):
                nc.gpsimd.memset(vaug[h][k][:, HD:HD + 1], 1.0)
        attn_n = [pers.tile([HD, S], bf, name=f"an{h}", tag=f"an{h}") for h in range(HPC)]

        with tc.tile_pool(name="xTp", bufs=1) as xpool, \
             tc.tile_pool(name="pjps", bufs=4, space="PSUM") as pjps:
            xT_t = []
            for i in range(NCT):
                t = xpool.tile([128, S], bf, name=f"xT{i}", tag=f"xT{i}")
                nc.sync.dma_start(out=t, in_=xT[128 * i:128 * (i + 1), :])
                xT_t.append(t)
            # q/k projections -> qkT[hh] [80, S] (transposed per head)
            for hh in range(2 * HPC):
                for c in range(NQC):
                    cs = slice(512 * c, 512 * (c + 1))
                    ps = pjps.tile([HD, 512], f32, name="qkps", tag="qkps")
                    for k in range(NCT):
                        nc.tensor.matmul(
                            ps, wqk_t[k][:, HD * hh:HD * (hh + 1)], xT_t[k][:, cs],
                            start=(k == 0), stop=(k == NCT - 1))
                    nc.scalar.copy(out=qkT[hh][:, cs], in_=ps)
            # v projection -> vaug[h][s][:, 0:80] (natural layout)
            for s in range(NKT):
                pv = pjps.tile([128, GDIM], f32, name="vps", tag="vps")
                for k in range(NCT):
                    nc.tensor.matmul(
                        pv, xT_t[k][:, 128 * s:128 * (s + 1)], wv_t[k],
                        start=(k == 0), stop=(k == NCT - 1))
                for h in range(HPC):
                    nc.vector.tensor_copy(out=vaug[h][s][:, 0:HD],
                                          in_=pv[:, HD * h:HD * (h + 1)])

        with tc.tile_pool(name="scps", bufs=3, space="PSUM") as scps, \
             tc.tile_pool(name="atps", bufs=2, space="PSUM") as atps, \
             tc.tile_pool(name="rbps", bufs=1, space="PSUM") as rbps, \
             tc.tile_pool(name="ops", bufs=2, space="PSUM") as ops, \
             tc.tile_pool(name="probs", bufs=2) as prpool, \
             tc.tile_pool(name="work", bufs=3) as work:
            for h in range(HPC):
                for c in range(NQC):
                    cs = slice(512 * c, 512 * (c + 1))
                    pbs = []
                    for k in range(NKT):
                        sp = scps.tile([128, 512], f32, name="sc", tag="sc")
                        nc.tensor.matmul(sp, qkT[HPC + h][:, 128 * k:128 * (k + 1)],
                                         qkT[h][:, cs], start=True, stop=True)
                        pb = prpool.tile([128, 512], bf, name=f"pb{k}", tag=f"pb{k}")
                        nc.scalar.activation(out=pb, in_=sp, func=Exp, scale=SM_SCALE)
                        pbs.append(pb)
                    ap_ = atps.tile([HD + 1, 512], f32, name="at", tag="at")
                    for k in range(NKT):
                        nc.tensor.matmul(ap_, vaug[h][k], pbs[k],
                                         start=(k == 0), stop=(k == NKT - 1))
                    # normalize: row 80 of ap_ holds the softmax denominators.
                    # Engines can't address a base partition of 80 (not
                    # 32-aligned) and DMA can't read PSUM, so: copy the whole
                    # [81,512] psum tile to SBUF (base 0 — legal), DMA row 80
                    # to partition 0, reciprocal, PE-broadcast, multiply.
                    au = work.tile([HD + 1, 512], f32, name="au", tag="au")
                    nc.vector.tensor_copy(out=au, in_=ap_)
                    rd0 = work.tile([1, 512], f32, name="rd0", tag="rd0")
                    nc.sync.dma_start(out=rd0, in_=au[HD:HD + 1, :])
                    rdr = work.tile([1, 512], bf, name="rdr", tag="rdr")
                    with nc.allow_low_precision(reason="softmax denom reciprocal; bf16 factor is plenty"):
                        nc.vector.reciprocal(out=rdr, in_=rd0)
                    rbc = rbps.tile([HD, 512], f32, name="rb", tag="rb")
                    nc.tensor.matmul(rbc, ones_bf, rdr, start=True, stop=True)
                    nc.vector.tensor_mul(out=attn_n[h][:, cs], in0=au[0:HD, :], in1=rbc)
            # partial output projection (contraction over this core's 320 dims)
            for dt_ in range(NCT):
                for c in range(NQC):
                    cs = slice(512 * c, 512 * (c + 1))
                    po = ops.tile([128, 512], f32, name="opo", tag="opo")
                    for h in range(HPC):
                        nc.tensor.matmul(po, wo_t[h][:, 128 * dt_:128 * (dt_ + 1)],
                                         attn_n[h][:, cs],
                                         start=(h == 0), stop=(h == HPC - 1))
                    ob = work.tile([128, 512], f32, name="ob", tag="ob")
                    nc.vector.tensor_copy(out=ob, in_=po)
                    nc.sync.dma_start(out=outT[128 * dt_:128 * (dt_ + 1), cs], in_=ob)


def build_nc(loop=1):
    import concourse.mybir as mybir
    import concourse.tile as tile
    from concourse import bacc

    bf = mybir.dt.bfloat16
    f32 = mybir.dt.float32
    nc = bacc.Bacc("TRN2", target_bir_lowering=False, debug=False,
                   num_devices=NCORES)
    xT = nc.dram_tensor("xT", [D, S], bf, kind="ExternalInput").ap()
    w_qk = nc.dram_tensor("w_qk", [D, 2 * GDIM], bf, kind="ExternalInput").ap()
    w_v = nc.dram_tensor("w_v", [D, GDIM], bf, kind="ExternalInput").ap()
    w_o = nc.dram_tensor("w_o", [GDIM, D], bf, kind="ExternalInput").ap()
    outT = nc.dram_tensor("outT", [D, S], f32, kind="ExternalOutput").ap()
    with tile.TileContext(nc) as tc:
        if loop == 1:
            _body(tc, xT, w_qk, w_v, w_o, outT)
        else:
            with tc.For_i(0, loop, 1):
                _body(tc, xT, w_qk, w_v, w_o, outT)
    nc.compile()
    return nc


def make_in_maps(inputs):
    """Host-side shard + layout prep. inputs: full-size fp32 arrays."""
    f = {k: np.asarray(v, dtype=np.float64) for k, v in inputs.items()}
    w_eff = {}
    for nm in ("q", "k", "v", "o"):
        w_eff[nm] = (f[f"w{nm}"] + f[f"{nm}_up"] @ f[f"{nm}_down"])
    bfd = ml_dtypes.bfloat16
    x = f["hidden_states"]  # [B, S, D]
    in_maps = []
    for c in range(NCORES):
        b, g = divmod(c, 2)
        rows = slice(GDIM * g, GDIM * (g + 1))
        xT = np.ascontiguousarray(x[b].T).astype(bfd)
        wq = w_eff["q"][rows, :].T  # [640, 320]
        wk = w_eff["k"][rows, :].T
        w_qk = np.ascontiguousarray(np.concatenate([wq, wk], axis=1)).astype(bfd)
        w_v = np.ascontiguousarray(w_eff["v"][rows, :].T).astype(bfd)
        w_o = np.ascontiguousarray(w_eff["o"][:, rows].T).astype(bfd)
        in_maps.append({"xT": xT, "w_qk": w_qk, "w_v": w_v, "w_o": w_o})
    return in_maps


def assemble_out(results, bo):
    out = np.empty((B, S, D), np.float32)
    for b in range(B):
        pt = results[2 * b]["outT"] + results[2 * b + 1]["outT"]  # [640, 2048]
        out[b] = pt.T + bo[None, :].astype(np.float32)
    return out


def kernel(**inputs):
    from concourse.bass_utils import run_bass_kernel_spmd

    if "nc" not in _cache:
        _cache["nc"] = build_nc()
    nc = _cache["nc"]
    in_maps = make_in_maps(inputs)
    res = run_bass_kernel_spmd(nc, in_maps, list(range(NCORES)))
    return assemble_out(res.results, np.asarray(inputs["bo"], np.float32))
